# revision 78
# baseline (speedup 1.0000x reference)
"""Trainium2 Bass kernel for nn_Attention_kv (dense transformer block).

Sharding: data-parallel over batch B=8 across the 8 NeuronCores — one batch
element per core, no collectives.

Structural optimizations vs the dense reference:

1. Mask compaction (host): ~50% of positions are masked; every masked QUERY
   row's final output equals ONE shared row per batch element:
       out_masked[b] = (mean_m text_x[b,m] @ Wkv[:,C:] + bkv[C:]) @ Wffn + bffn
   (uniform softmax -> mean of cross-attn v; mean commutes with the linear
   maps). Valid rows attend only to valid keys. The host gathers valid rows
   (padded to static NV=576 >= observed max counts 534/547; overflow falls
   back to a host compute), the device runs a 576-token pipeline, the host
   scatters and fills masked rows.

2. Projection fusion (host algebra): S1 = (xWq)(xWk)^T = x (Wq_s Wk^T) x^T,
   so q/k projections collapse to ONE t1 = x @ Wqk and the raw x^T serves as
   keys; likewise S2 = o1 (Wcq_s Wck^T) t^T. The ffn is folded into the
   cross-attn value path: out = P2 @ (t (Wcv Wffn)) + bias. Bias pieces fold
   into projection biases, cancel under softmax (per-query), or join the
   per-key mask bias colb (host-computed). Softmax scale pre-folded.

3. bf16 PE datapath (1 cycle/row at any width; fp32 PSUM accumulation),
   host pre-transposed/pre-laid-out inputs ([P, a, n] so every DMA row is
   contiguous), single-queue DMA prefetch in exact first-need order.

Per-core pipeline (NV=576, C=768, [part, free] layouts):
  t1 = x@Wqk (^T layout) and v projections
  -> attn1 transposed-scores flash: S^T tile -> exp(S^T + colb1) fused on
     the scalar engine; attn@v accumulated over key tiles in 6 PSUM banks;
     rowsums via ones-matmul; normalization at PSUM->SBUF copyback, its
     recip->bcast->mul tail split into closures spread across later PE work
  -> t2 = o1@Wqck -> cvf = t@(Wcv Wffn)
  -> attn2 fused with ffn: natural-layout output accumulated directly from
     probability tiles against cvf; per-query normalization via PE-transposed
     reciprocal columns (per-partition scalar); og written per subtile.
"""

import sys

sys.path.insert(0, "/opt/trn_rl_repo")

from contextlib import ExitStack

import numpy as np
import ml_dtypes

import concourse.bass as bass
import concourse.mybir as mybir
import concourse.tile as tile
from concourse import bacc
from concourse.bass_utils import run_bass_kernel_spmd
from concourse.masks import make_identity

P = 128
M = 1024  # full sequence length per batch element
C = 768  # model dim
KT = C // P  # 6 contraction tiles
NV = 576  # compacted valid seq len; covers observed max counts 534 (cpu-jax)
# and 547 (axon-jax) with margin; host fallback handles any overflow
NT = 5  # seq tiles: 4 full + one 64-row tail
TILES = [(0, 128), (128, 128), (256, 128), (384, 128), (512, 64)]
FCH = 288  # query free chunk
NCH = NV // FCH  # 2
SCALE = float(C) ** -0.5

F32 = mybir.dt.float32
F32R = mybir.dt.float32r
BF16 = mybir.dt.bfloat16
AF = mybir.ActivationFunctionType
AL = mybir.AluOpType
BF16_NP = ml_dtypes.bfloat16

N_CORES = 8


def _proj_T(nc, psum, dst, w_s, src, bcol, nm, defer=None, c_outer=False,
            qchunks=None, psum_first=None, n_first=0):
    """dst[:, d, :] ([P, KT, NV] bf16) = (src-cols @ W)^T + bias.

    w_s: [P, KT_d, KT_a, P] weight (lhsT tiles [128 contract, 128 out-dim])
    src: [P, KT, NV] activations^T (rhs tiles, contract on partitions)
    bcol: [P, KT] per-out-dim bias columns
    defer: list of closures, one emitted after each matmul group (hides a
    preceding phase's recip->bcast chain behind this phase's PE work)
    c_outer: emit all d-groups of chunk 0 before touching chunk 1 -- use when
    the src's later chunks are produced by the deferred closure
    qchunks: override the free-dim chunk list [(off, w), ...]
    psum_first/n_first: allocate the first n groups' psum from this pool's
    "st" ring instead -- after an attention, the "po" ring's next slots are
    still gated on that attention's normalization chain
    """
    if qchunks is None:
        qchunks = [(c * FCH, FCH) for c in range(NCH)]
    order = (
        [(d, c) for c in range(len(qchunks)) for d in range(KT)]
        if c_outer
        else [(d, c) for d in range(KT) for c in range(len(qchunks))]
    )
    defer = list(defer) if defer else []
    for gi, (d, c) in enumerate(order):
        off, w = qchunks[c]
        if gi < n_first:
            ps = psum_first.tile([P, 512], F32, tag="st", name=f"ps_{nm}_{d}_{c}")
        else:
            ps = psum.tile([P, 512], F32, tag="po", name=f"ps_{nm}_{d}_{c}")
        for a in range(KT):
            nc.tensor.matmul(
                ps[:, :w],
                w_s[:, d, a, :],
                src[:, a, off : off + w],
                start=(a == 0),
                stop=(a == KT - 1),
            )
        if defer:
            defer.pop(0)()
        nc.vector.tensor_scalar_add(
            dst[:, d, off : off + w], ps[:, :w], bcol[:, d : d + 1]
        )


def _proj_nat(nc, psum, dst, w_s, src, bias_bc, nm):
    """dst[:, i, :] ([P, NT, C] bf16) = src-rows @ W + bias (natural layout).

    src: [P, KT, NV] activations^T -- lhsT tiles [128 contract, 128 seq]
    w_s: [P, KT, C] weight (rhs, contract on partitions)
    bias_bc: [P, C] broadcast bias
    """
    chunks = [(0, 512), (512, 256)]
    for i, (ioff, ih) in enumerate(TILES):
        pss = []
        for (off, w) in chunks:
            ps = psum.tile([P, 512], F32, tag="po", name=f"ps_{nm}_{i}_{off}")
            for a in range(KT):
                nc.tensor.matmul(
                    ps[:ih, :w],
                    src[:, a, ioff : ioff + ih],
                    w_s[:, a, off : off + w],
                    start=(a == 0),
                    stop=(a == KT - 1),
                )
            pss.append(ps)
        for ci, ((off, w), ps) in enumerate(zip(chunks, pss)):
            eng = nc.vector
            eng.tensor_add(out=dst[:ih, i, off : off + w], in0=ps[:ih, :w], in1=bias_bc[:ih, off : off + w])


def _attention(nc, io, psum_main, psum_att, qT, kT, vn, oT, colb, ones_r, ones_row_r, label):
    """oT[:, d, :] = normalized masked-softmax attention output^T ([P, KT, NV] bf16).

    qT, kT: [P, KT, NV] bf16 (d on partitions; scale pre-folded into q).
    vn: [P, NT, C] bf16 (natural).
    colb: [P, NT] f32 = (kmask-1)*10000 along sk partitions (kills pad keys).

    Each chunk's normalization tail (recip bcast matmul + PSUM->SBUF
    copybacks) is DEFERRED and split into parts, emitted one part per
    subsequent PE matmul group, so the PE never head-of-line blocks on the
    DVE recip and the DVE queue never gets one big batch that starves the
    PSUM-ring copybacks. Returns the last chunk's tail parts for the caller
    to spread inside the next phase (via _proj_T/ffn `defer`).
    """
    pend = []
    for c in range(NCH):
        sq = slice(c * FCH, (c + 1) * FCH)
        pos = [
            psum_att.tile([P, FCH], F32, tag="po", name=f"po_{label}_{c}_{d}")
            for d in range(KT)
        ]
        p_tiles = []
        pending_av = []  # av matmuls lag scores by 2 key-tiles so the
        # previous chunk's deferred tail (DVE/Pool copybacks freeing the po
        # banks) completes off the PE critical path

        def av_flush(jj):
            pp = p_tiles[jj]
            jh = TILES[jj][1]
            for d in range(KT):
                nc.tensor.matmul(
                    pos[d][:],
                    vn[:jh, jj, d * P : (d + 1) * P],
                    pp[:jh, :],
                    start=(jj == 0),
                    stop=(jj == NT - 1),
                )

        for j, (joff, jh) in enumerate(TILES):
            st = psum_main.tile([P, 512], F32, tag="st", name=f"st_{label}_{c}_{j}")
            for a in range(KT):
                nc.tensor.matmul(
                    st[:jh, :FCH],
                    kT[:, a, joff : joff + jh],
                    qT[:, a, sq],
                    start=(a == 0),
                    stop=(a == KT - 1),
                )
            if pend:
                pend.pop(0)()
            pj = io.tile([P, FCH], BF16, tag="pp", name=f"p_{label}_{c}_{j}", bufs=NT + 3)
            nc.scalar.activation(pj[:jh, :], st[:jh, :FCH], AF.Exp, bias=colb[:jh, j : j + 1])
            p_tiles.append(pj)
            pending_av.append(j)
            if len(pending_av) > 2:
                av_flush(pending_av.pop(0))
        for jj in pending_av:
            av_flush(jj)
        # row sums over sk (partitions + tiles) via ones-matmul
        rs = psum_main.tile([P, 512], F32, tag="st", name=f"rs_{label}_{c}")
        for j, (joff, jh) in enumerate(TILES):
            nc.tensor.matmul(
                rs[0:1, :FCH],
                ones_r[:jh, :],
                p_tiles[j][:jh, :],
                start=(j == 0),
                stop=(j == NT - 1),
            )
        recip = io.tile([1, FCH], F32R, tag="recip", name=f"recip_{label}_{c}", bufs=2)
        with nc.allow_low_precision(reason="f32r recip feeds f32r bcast matmul"):
            nc.vector.reciprocal(recip[:], rs[0:1, :FCH])

        rbc_box = []

        def tail_bcast(recip=recip, c=c, rbc_box=rbc_box):
            bc = psum_main.tile([P, 512], F32, tag="st", name=f"bc_{label}_{c}")
            nc.tensor.matmul(bc[:, :FCH], ones_row_r[:], recip[:], start=True, stop=True)
            rbc = io.tile([P, FCH], F32, tag="rbc", name=f"rbc_{label}_{c}", bufs=2)
            nc.vector.tensor_copy(out=rbc[:], in_=bc[:, :FCH])
            rbc_box.append(rbc)

        def tail_muls(ds, sq=sq, pos=pos, rbc_box=rbc_box):
            for d in ds:
                nc.vector.tensor_mul(out=oT[:, d, sq], in0=pos[d][:], in1=rbc_box[0][:])

        pend = [tail_bcast] + [
            (lambda ds=ds: tail_muls(ds)) for ds in [(0, 1), (2, 3), (4, 5)]
        ]
    return pend


A2CH = [(0, 256), (256, 256), (512, 64)]  # attn2 query chunks (128-aligned)
A2SUB = [(0, 128), (128, 128), (256, 128), (384, 128), (512, 64)]  # out subtiles


def _attention2_fused(nc, io, psum_main, psum_att, qT, kT, cvf, og_d, colb,
                      rcol, ones_r, ident, bout_bc, defer):
    """Fused attention-2 + ffn: og[q, :] = softmax2(q) @ cvf + bout.

    cvf = t @ (Wcv Wffn) so the attn@v accumulation directly produces the
    final output in NATURAL layout [q part, d' free]; the per-query softmax
    normalization is then a per-partition scalar (rcol), obtained by
    PE-transposing the reciprocal row -- no broadcast matmul, no ffn phase.
    av groups for chunk c are spread across chunk c+1's scores slots.
    """
    p_store = {}  # (c, j) -> p2 tile
    av_queue = []  # pending (c, subtile) av emissions
    fch = [(0, 512), (512, 256)]
    defer = list(defer) if defer else []

    def av_emit(c, t, lo, tw):
        # both free-chunks of one output subtile -> one fin tile, one DMA
        fin = io.tile([P, C], BF16, tag="fin", name=f"fin_{t}", bufs=NT)
        for (off, w) in fch:
            ps = psum_att.tile([P, 512], F32, tag="po", name=f"av2_{t}_{off}")
            for j, (joff, jh) in enumerate(TILES):
                nc.tensor.matmul(
                    ps[:tw, :w],
                    p_store[(c, j)][:jh, lo : lo + tw],
                    cvf[:jh, j, off : off + w],
                    start=(j == 0),
                    stop=(j == NT - 1),
                )
            nc.vector.scalar_tensor_tensor(
                out=fin[:tw, off : off + w],
                in0=ps[:tw, :w],
                scalar=rcol[:tw, t : t + 1],
                in1=bout_bc[:tw, off : off + w],
                op0=AL.mult,
                op1=AL.add,
            )
        eng = nc.scalar if t % 2 == 0 else nc.sync
        eng.dma_start(og_d[t * P : t * P + tw, :], fin[:tw, :])

    def av_pop():
        if av_queue:
            av_queue.pop(0)()

    for c, (qoff, qw) in enumerate(A2CH):
        for j, (joff, jh) in enumerate(TILES):
            st = psum_main.tile([P, 512], F32, tag="st", name=f"st_a2_{c}_{j}")
            for a in range(KT):
                nc.tensor.matmul(
                    st[:jh, :qw],
                    kT[:, a, joff : joff + jh],
                    qT[:, a, qoff : qoff + qw],
                    start=(a == 0),
                    stop=(a == KT - 1),
                )
            if defer:
                defer.pop(0)()
            else:
                av_pop()
            pj = io.tile([P, 256], BF16, tag="pp2", name=f"p2_{c}_{j}", bufs=12)
            nc.scalar.activation(pj[:jh, :qw], st[:jh, :qw], AF.Exp, bias=colb[:jh, j : j + 1])
            p_store[(c, j)] = pj
        # rowsum over keys -> reciprocal row -> PE-transpose to column layout
        rs = psum_main.tile([P, 512], F32, tag="st", name=f"rs_a2_{c}")
        for j, (joff, jh) in enumerate(TILES):
            nc.tensor.matmul(
                rs[0:1, :qw],
                ones_r[:jh, :],
                p_store[(c, j)][:jh, :qw],
                start=(j == 0),
                stop=(j == NT - 1),
            )
        rrow = io.tile([1, 256], F32, tag="rrow", name=f"rrow_{c}", bufs=2)
        nc.vector.reciprocal(rrow[0:1, :qw], rs[0:1, :qw])
        rc_ps = psum_main.tile([P, 512], F32, tag="st", name=f"rcps_{c}")
        subs = [(t, lo, tw) for (t, (g, tw)) in enumerate(A2SUB)
                for lo in [g - qoff] if 0 <= lo < qw]
        for si, (t, lo, tw) in enumerate(subs):
            nc.tensor.transpose(rc_ps[:tw, si : si + 1], rrow[0:1, lo : lo + tw], ident[0:1, 0:1])
        for si, (t, lo, tw) in enumerate(subs):
            nc.vector.tensor_copy(out=rcol[:tw, t : t + 1], in_=rc_ps[:tw, si : si + 1])
        for (t, lo, tw) in subs:
            av_queue.append(lambda c=c, t=t, lo=lo, tw=tw: av_emit(c, t, lo, tw))
    while av_queue:
        av_pop()


def build_nc(n_iters=1):
    nc = bacc.Bacc(trn_type="TRN2", target_bir_lowering=False, debug=False)

    # all big inputs arrive pre-arranged in SBUF layout (host does the
    # (a p) -> p a shuffles) so every DMA row is fully contiguous.
    # T-projection weights additionally carry the out-dim (d) outermost so
    # they can stream per-d-block: [P, d, a, 128].
    xgT_d = nc.dram_tensor("xgT", [P, KT, NV], BF16, kind="ExternalInput").ap()
    tgT_d = nc.dram_tensor("tgT", [P, KT, NV], BF16, kind="ExternalInput").ap()
    colb_ds = {
        nm: nc.dram_tensor(nm, [P, NT], F32, kind="ExternalInput").ap()
        for nm in ["colb1", "colb2"]
    }
    w_ds = {}
    for nm in ["wqk", "wqck"]:
        w_ds[nm] = nc.dram_tensor(nm, [P, KT, KT, P], BF16, kind="ExternalInput").ap()
    for nm in ["wv", "wcvf"]:
        w_ds[nm] = nc.dram_tensor(nm, [P, KT, C], BF16, kind="ExternalInput").ap()
    bcol_ds = {
        nm: nc.dram_tensor(nm, [P, KT], F32, kind="ExternalInput").ap()
        for nm in ["bqk", "bqck"]
    }
    brow_ds = {
        nm: nc.dram_tensor(nm, [1, C], F32, kind="ExternalInput").ap()
        for nm in ["bv", "bcvf", "bf"]
    }
    og_d = nc.dram_tensor("og", [NV, C], BF16, kind="ExternalOutput").ap()

    with tile.TileContext(nc) as tc, ExitStack() as ctx:
        const = ctx.enter_context(tc.tile_pool(name="const", bufs=1))
        acts = ctx.enter_context(tc.tile_pool(name="acts", bufs=1))
        wpool = ctx.enter_context(tc.tile_pool(name="wpool", bufs=1))
        io = ctx.enter_context(tc.tile_pool(name="io", bufs=1))
        psum_main = ctx.enter_context(tc.tile_pool(name="psum_main", bufs=2, space="PSUM"))
        psum_att = ctx.enter_context(tc.tile_pool(name="psum_att", bufs=6, space="PSUM"))

        # ---- constants ----
        ones32 = const.tile([P, 1], F32, tag="ones32", name="ones32")
        nc.gpsimd.memset(ones32[:], 1.0)
        ones_r = const.tile([P, 1], BF16, tag="ones_r", name="ones_r")
        nc.vector.tensor_copy(out=ones_r[:], in_=ones32[:])
        ones_row32 = const.tile([1, P], F32, tag="ones_row32", name="ones_row32")
        nc.gpsimd.memset(ones_row32[:], 1.0)
        ones_row_r = const.tile([1, P], F32R, tag="ones_row_r", name="ones_row_r")
        nc.vector.tensor_copy(out=ones_row_r[:], in_=ones_row32[:])

        # const tiles; their DMAs are issued inside the first body iteration,
        # sequenced behind the critical first weight loads
        colbs = {}
        for nm in ["colb1", "colb2"]:
            colbs[nm] = const.tile([P, NT], F32, tag=f"colb_{nm}", name=f"{nm}_s")
        bcols = {}
        for nm in ["bqk", "bqck"]:
            bcols[nm] = const.tile([P, KT], F32, tag=f"bcol_{nm}", name=f"bcol_{nm}")
        brows = {}
        for nm in ["bv", "bcvf", "bf"]:
            brows[nm] = const.tile([P, C], F32, tag=f"brow_{nm}", name=f"brow_{nm}")
        ident32 = const.tile([P, P], F32, tag="ident32", name="ident32")
        make_identity(nc, ident32[:])

        # weight tiles resident in SBUF for the whole kernel; DMAs are issued
        # inside the first body iteration, interleaved in first-use order
        w_ss = {}
        for nm in ["wqk", "wqck"]:
            w_ss[nm] = wpool.tile([P, KT, KT, P], BF16, tag=f"w_{nm}", name=f"ws_{nm}")
        for nm in ["wv", "wcvf"]:
            w_ss[nm] = wpool.tile([P, KT, C], BF16, tag=f"w_{nm}", name=f"ws_{nm}")

        for _it in range(n_iters):
            _body_iter(nc, tc, acts, io, psum_main, psum_att,
                       xgT_d, tgT_d, og_d, w_ds, w_ss, bcols, brows, colbs,
                       bcol_ds, brow_ds, colb_ds, ones_r, ones_row_r, ident32, _it)

    nc.compile()
    return nc


def _body_iter(nc, tc, acts, io, psum_main, psum_att,
               xgT_d, tgT_d, og_d, w_ds, w_ss, bcols, brows, colbs,
               bcol_ds, brow_ds, colb_ds, ones_r, ones_row_r, ident32, it):
    xgT = acts.tile([P, KT, NV], BF16, tag="xgT", name="xgT")
    tgT = acts.tile([P, KT, NV], BF16, tag="tgT", name="tgT")
    if it == 0:
        # single-queue prefetch in exact first-need order; first tiles split
        # so the first projection matmuls start as early as possible
        nc.sync.dma_start(xgT[:, :, :128], xgT_d[:, :, :128])
        nc.sync.dma_start(w_ss["wqk"][:, 0:1], w_ds["wqk"][:, 0:1])
        nc.sync.dma_start(xgT[:, :, 128:FCH], xgT_d[:, :, 128:FCH])
        nc.sync.dma_start(bcols["bqk"][:], bcol_ds["bqk"][:])
        nc.sync.dma_start(xgT[:, :, FCH:], xgT_d[:, :, FCH:])
        nc.sync.dma_start(w_ss["wqk"][:, 1:3], w_ds["wqk"][:, 1:3])
        nc.sync.dma_start(w_ss["wqk"][:, 3:6], w_ds["wqk"][:, 3:6])
        nc.sync.dma_start(w_ss["wv"][:], w_ds["wv"][:])
        nc.sync.dma_start(brows["bv"][:], brow_ds["bv"][0:1, :].partition_broadcast(P))
        nc.sync.dma_start(colbs["colb1"][:], colb_ds["colb1"][:])
        nc.sync.dma_start(tgT[:], tgT_d[:])
        nc.sync.dma_start(w_ss["wqck"][:], w_ds["wqck"][:])
        nc.sync.dma_start(bcols["bqck"][:], bcol_ds["bqck"][:])
        nc.sync.dma_start(colbs["colb2"][:], colb_ds["colb2"][:])
        nc.sync.dma_start(w_ss["wcvf"][:], w_ds["wcvf"][:])
        nc.sync.dma_start(brows["bcvf"][:], brow_ds["bcvf"][0:1, :].partition_broadcast(P))
        nc.sync.dma_start(brows["bf"][:], brow_ds["bf"][0:1, :].partition_broadcast(P))
    else:
        nc.sync.dma_start(xgT[:], xgT_d[:])
        nc.sync.dma_start(tgT[:], tgT_d[:])

    # fused score weights: S1 = x @ (Wq*s @ Wk^T) @ x^T, so attention-1
    # consumes t1 = x @ Wqk as queries and the raw xgT as keys; likewise
    # S2 = o1 @ (Wq*s @ Wck^T) @ t^T. The q/k/cq/ck projections collapse
    # into one projection per attention. Bias terms: per-dim parts fold into
    # bqk/bqck; per-query parts cancel under softmax; per-key parts are
    # folded into colb1/colb2 on the host.
    t1T = acts.tile([P, KT, NV], BF16, tag="qT", name="t1T")
    vn = acts.tile([P, NT, C], BF16, tag="vn", name="vn")
    o1T = acts.tile([P, KT, NV], BF16, tag="oT", name="o1T")

    # ---- phase 1: t1/v projections ----
    # t1 consumes xgT in three pieces matching the DMA arrival order so the
    # first matmul starts after only ~0.5 MB has landed
    _proj_T(nc, psum_att, t1T, w_ss["wqk"], xgT, bcols["bqk"], "t1",
            qchunks=[(0, 128), (128, 160), (288, 288)])
    _proj_nat(nc, psum_att, vn, w_ss["wv"], xgT, brows["bv"], "v")

    # ---- phase 2: attention 1 (keys = raw xgT) ----
    a1_tail = _attention(nc, io, psum_main, psum_att, t1T, xgT, vn, o1T,
                         colbs["colb1"], ones_r, ones_row_r, "a1")

    # ---- phase 3: t2 projection (reuses t1T slot) ----
    t2T = acts.tile([P, KT, NV], BF16, tag="qT", name="t2T")
    _proj_T(nc, psum_att, t2T, w_ss["wqck"], o1T, bcols["bqck"], "t2",
            defer=a1_tail, c_outer=True, psum_first=psum_main, n_first=2)

    # ---- phase 4: cvf projection from text (reuses vn slot) ----
    # cvf = t @ (Wcv Wffn): the ffn is folded into the cross-attn value path
    cvf = acts.tile([P, NT, C], BF16, tag="vn", name="cvf")
    _proj_nat(nc, psum_att, cvf, w_ss["wcvf"], tgT, brows["bcvf"], "cvf")

    # ---- phase 5: fused attention 2 + ffn -> og ----
    rcol = io.tile([P, NT], F32, tag="rcol", name="rcol", bufs=2)
    _attention2_fused(nc, io, psum_main, psum_att, t2T, tgT, cvf, og_d,
                      colbs["colb2"], rcol, ones_r, ident32, brows["bf"],
                      None)


# ---------------- host side ----------------

_NC_CACHE = None


def _get_nc():
    global _NC_CACHE
    if _NC_CACHE is None:
        _NC_CACHE = build_nc()
    return _NC_CACHE


def prepare_static(Wqkv, bqkv, Wq, bq, Wkv, bkv, Wffn, bffn):
    """Shared (per-call, batch-independent) device inputs."""
    s = np.float32(SCALE)
    f32 = np.float32

    def bf(a):  # [C, N] -> [P, KT, N] bf16 with [p, a_, n] = arr[a_*P + p, n]
        a = np.asarray(a)
        return np.ascontiguousarray(
            a.reshape(KT, P, a.shape[1]).transpose(1, 0, 2)
        ).astype(BF16_NP)

    def bf4(a):  # [C, C] -> [P, KT_d, KT_a, P] with [p, d, a_, j] = arr[a_*P+p, d*P+j]
        a = np.asarray(a)
        return np.ascontiguousarray(
            a.reshape(KT, P, KT, P).transpose(1, 2, 0, 3)
        ).astype(BF16_NP)

    def col(b):  # [C] -> [P, KT] with [p, a] = b[a*P + p]
        return np.ascontiguousarray(np.asarray(b, f32).reshape(KT, P).T)

    f64 = np.float64
    wq_s = Wqkv[:, :C].astype(f64) * float(SCALE)
    wk = Wqkv[:, C : 2 * C].astype(f64)
    wcq_s = Wq.astype(f64) * float(SCALE)
    wck = Wkv[:, :C].astype(f64)
    bq1_s = bqkv[:C].astype(f64) * float(SCALE)
    bq2_s = bq.astype(f64) * float(SCALE)

    wf64 = Wffn.astype(f64)
    static = {
        # fused score weights: S1 = x (Wq_s Wk^T) x^T, S2 = o1 (Wcq_s Wck^T) t^T
        "wqk": bf4(wq_s @ wk.T),
        "wqck": bf4(wcq_s @ wck.T),
        "wv": bf(Wqkv[:, 2 * C :]),
        # ffn folded into the cross-attn value path: out = P2 @ (t Wcv Wf) + ...
        "wcvf": bf(Wkv[:, C:].astype(f64) @ wf64),
        # per-dim bias parts of the fused projections
        "bqk": col(bq1_s @ wk.T),
        "bqck": col(bq2_s @ wck.T),
        "bv": np.ascontiguousarray(bqkv[2 * C :], f32).reshape(1, C),
        "bcvf": np.ascontiguousarray(bkv[C:].astype(f64) @ wf64, f32).reshape(1, C),
        "bf": np.ascontiguousarray(bffn, f32).reshape(1, C),
    }
    # per-key score bias directions (keys @ wtilde added to colb on the host;
    # the per-query counterparts cancel under softmax)
    aux = {
        "wt1": (wk @ bq1_s).astype(f32),  # attn1 keys are x rows
        "wt2": (wck @ bq2_s).astype(f32),  # attn2 keys are text rows
    }
    return static, aux


def prepare_core(layout_xb, text_xb, maskb, aux):
    """Per-batch-element compacted device inputs. Returns (in_map, idx) or
    (None, None) if the valid count exceeds NV (host fallback)."""
    idx = np.flatnonzero(maskb != 0)
    nv = len(idx)
    if nv > NV:
        return None, None
    pad_to = idx[0] if nv > 0 else 0
    idxp = np.concatenate([idx, np.full(NV - nv, pad_to, dtype=idx.dtype)])
    km = np.zeros(NT * P, np.float32)  # padded past NV for the colb reshape
    km[:nv] = 1.0
    xg = layout_xb[idxp]
    tg = text_xb[idxp]

    def xf(a):  # [NV, C] gathered rows -> [P, KT, NV] bf16 transposed layout
        return np.ascontiguousarray(
            a.T.reshape(KT, P, NV).transpose(1, 0, 2)
        ).astype(BF16_NP)

    def colb(beta):  # per-key additive score bias incl. pad-kill mask
        v = (km - 1.0) * 10000.0
        v[:NV] += beta
        return np.ascontiguousarray(v.reshape(NT, P).T)

    in_map = {
        "xgT": xf(xg),
        "tgT": xf(tg),
        "colb1": colb(xg.astype(np.float32) @ aux["wt1"]),
        "colb2": colb(tg.astype(np.float32) @ aux["wt2"]),
    }
    return in_map, idx


def masked_row(text_xb, Wkv, bkv, Wffn, bffn):
    """The shared final-output row for all masked positions of one batch
    element: uniform attention over ALL keys -> mean of cross-attn v."""
    mt = text_xb.astype(np.float64).mean(axis=0)
    mcv = mt @ Wkv[:, C:].astype(np.float64) + bkv[C:].astype(np.float64)
    return (mcv @ Wffn.astype(np.float64) + bffn.astype(np.float64)).astype(np.float32)


def _numpy_ref_one(x, t, mask, Wqkv, bqkv, Wq, bq, Wkv, bkv, Wffn, bffn):
    """f64 reference for one batch element (fallback if nv > NV)."""
    x = x.astype(np.float64)
    t = t.astype(np.float64)
    mask = mask.astype(np.float64)
    pair = (mask[:, None] * mask[None, :]) != 0
    scale = C ** -0.5

    def attn(q, k, v):
        sM = (q @ k.T) * scale
        sM = np.where(pair, sM, -10000.0)
        sM = sM - sM.max(axis=-1, keepdims=True)
        e = np.exp(sM)
        return (e / e.sum(axis=-1, keepdims=True)) @ v

    qkv = x @ Wqkv.astype(np.float64) + bqkv.astype(np.float64)
    q, k, v = np.split(qkv, 3, axis=-1)
    lo = attn(q, k, v)
    cq = lo @ Wq.astype(np.float64) + bq.astype(np.float64)
    kv = t @ Wkv.astype(np.float64) + bkv.astype(np.float64)
    ck, cv = np.split(kv, 2, axis=-1)
    mg = attn(cq, ck, cv)
    return (mg @ Wffn.astype(np.float64) + bffn.astype(np.float64)).astype(np.float32)


def kernel(layout_x, text_x, mask, Wqkv, bqkv, Wq, bq, Wkv, bkv, Wffn, bffn):
    layout_x = np.ascontiguousarray(np.asarray(layout_x, dtype=np.float32))
    text_x = np.ascontiguousarray(np.asarray(text_x, dtype=np.float32))
    mask = np.ascontiguousarray(np.asarray(mask, dtype=np.float32))
    Wqkv = np.ascontiguousarray(np.asarray(Wqkv, dtype=np.float32))
    bqkv = np.ascontiguousarray(np.asarray(bqkv, dtype=np.float32)).reshape(3 * C)
    Wq = np.ascontiguousarray(np.asarray(Wq, dtype=np.float32))
    bq = np.ascontiguousarray(np.asarray(bq, dtype=np.float32)).reshape(C)
    Wkv = np.ascontiguousarray(np.asarray(Wkv, dtype=np.float32))
    bkv = np.ascontiguousarray(np.asarray(bkv, dtype=np.float32)).reshape(2 * C)
    Wffn = np.ascontiguousarray(np.asarray(Wffn, dtype=np.float32))
    bffn = np.ascontiguousarray(np.asarray(bffn, dtype=np.float32)).reshape(C)

    B = layout_x.shape[0]
    assert B == N_CORES

    static, aux = prepare_static(Wqkv, bqkv, Wq, bq, Wkv, bkv, Wffn, bffn)
    in_maps, idxs = [], []
    fallback = {}
    for b in range(B):
        in_map, idx = prepare_core(layout_x[b], text_x[b], mask[b], aux)
        if in_map is None:
            fallback[b] = _numpy_ref_one(
                layout_x[b], text_x[b], mask[b],
                Wqkv, bqkv, Wq, bq, Wkv, bkv, Wffn, bffn,
            )
            in_map, idx = prepare_core(
                np.zeros_like(layout_x[b]), np.zeros_like(text_x[b]),
                np.zeros(M, np.float32), aux,
            )
        in_maps.append({**in_map, **static})
        idxs.append(idx)

    nc = _get_nc()
    res = run_bass_kernel_spmd(nc, in_maps, core_ids=list(range(N_CORES)))

    out = np.empty((B, M, C), np.float32)
    for b in range(B):
        if b in fallback:
            out[b] = fallback[b]
            continue
        mrow = masked_row(text_x[b], Wkv, bkv, Wffn, bffn)
        out[b][:] = mrow[None, :]
        idx = idxs[b]
        if len(idx):
            out[b][idx] = res.results[b]["og"][: len(idx)].astype(np.float32)
    return out


# revision 83
# speedup vs baseline: 1.4021x; 1.4021x over previous
"""Trainium2 Bass kernel for nn_Attention_kv (dense transformer block).

Sharding: data-parallel over batch B=8 across the 8 NeuronCores — one batch
element per core, no collectives.

Structural optimizations vs the dense reference:

1. Mask compaction (host): ~50% of positions are masked; every masked QUERY
   row's final output equals ONE shared row per batch element:
       out_masked[b] = (mean_m text_x[b,m] @ Wkv[:,C:] + bkv[C:]) @ Wffn + bffn
   (uniform softmax -> mean of cross-attn v; mean commutes with the linear
   maps). Valid rows attend only to valid keys. The host gathers valid rows
   (padded to static NV=576 >= observed max counts 534/547; overflow falls
   back to a host compute), the device runs a 576-token pipeline, the host
   scatters and fills masked rows.

2. Projection fusion (host algebra): S1 = (xWq)(xWk)^T = x (Wq_s Wk^T) x^T,
   so q/k projections collapse to ONE t1 = x @ Wqk and the raw x^T serves as
   keys; likewise S2 = o1 (Wcq_s Wck^T) t^T. The ffn is folded into the
   cross-attn value path: out = P2 @ (t (Wcv Wffn)) + bias. Bias pieces fold
   into projection biases, cancel under softmax (per-query), or join the
   per-key mask bias colb (host-computed). Softmax scale pre-folded.

3. bf16 PE datapath (1 cycle/row at any width; fp32 PSUM accumulation),
   host pre-transposed/pre-laid-out inputs ([P, a, n] so every DMA row is
   contiguous), single-queue DMA prefetch in exact first-need order.

Per-core pipeline (NV=576, C=768, [part, free] layouts):
  t1 = x@Wqk (^T layout) and v projections
  -> attn1 transposed-scores flash: S^T tile -> exp(S^T + colb1) fused on
     the scalar engine; attn@v accumulated over key tiles in 6 PSUM banks;
     rowsums via ones-matmul; normalization at PSUM->SBUF copyback, its
     recip->bcast->mul tail split into closures spread across later PE work
  -> t2 = o1@Wqck -> cvf = t@(Wcv Wffn)
  -> attn2 fused with ffn: natural-layout output accumulated directly from
     probability tiles against cvf; per-query normalization via PE-transposed
     reciprocal columns (per-partition scalar); og written per subtile.
"""

import sys

sys.path.insert(0, "/opt/trn_rl_repo")

from contextlib import ExitStack

import numpy as np
import ml_dtypes

import concourse.bass as bass
import concourse.mybir as mybir
import concourse.tile as tile
from concourse import bacc
from concourse.bass_utils import run_bass_kernel_spmd
from concourse.masks import make_identity

P = 128
M = 1024  # full sequence length per batch element
C = 768  # model dim
KT = C // P  # 6 contraction tiles
NV = 576  # compacted valid seq len; covers observed max counts 534 (cpu-jax)
# and 547 (axon-jax) with margin; host fallback handles any overflow
NT = 5  # seq tiles: 4 full + one 64-row tail
TILES = [(0, 128), (128, 128), (256, 128), (384, 128), (512, 64)]
FCH = 288  # query free chunk
NCH = NV // FCH  # 2
SCALE = float(C) ** -0.5

F32 = mybir.dt.float32
F32R = mybir.dt.float32r
BF16 = mybir.dt.bfloat16
AF = mybir.ActivationFunctionType
AL = mybir.AluOpType
BF16_NP = ml_dtypes.bfloat16

N_CORES = 8


def _proj_T(nc, psum, dst, w_s, src, bcol, nm, defer=None, c_outer=False,
            qchunks=None, psum_first=None, n_first=0):
    """dst[:, d, :] ([P, KT, NV] bf16) = (src-cols @ W)^T + bias.

    w_s: [P, KT_d, KT_a, P] weight (lhsT tiles [128 contract, 128 out-dim])
    src: [P, KT, NV] activations^T (rhs tiles, contract on partitions)
    bcol: [P, KT] per-out-dim bias columns
    defer: list of closures, one emitted after each matmul group (hides a
    preceding phase's recip->bcast chain behind this phase's PE work)
    c_outer: emit all d-groups of chunk 0 before touching chunk 1 -- use when
    the src's later chunks are produced by the deferred closure
    qchunks: override the free-dim chunk list [(off, w), ...]
    psum_first/n_first: allocate the first n groups' psum from this pool's
    "st" ring instead -- after an attention, the "po" ring's next slots are
    still gated on that attention's normalization chain
    """
    if qchunks is None:
        qchunks = [(c * FCH, FCH) for c in range(NCH)]
    order = (
        [(d, c) for c in range(len(qchunks)) for d in range(KT)]
        if c_outer
        else [(d, c) for d in range(KT) for c in range(len(qchunks))]
    )
    defer = list(defer) if defer else []
    for gi, (d, c) in enumerate(order):
        off, w = qchunks[c]
        if gi < n_first:
            ps = psum_first.tile([P, 512], F32, tag="st", name=f"ps_{nm}_{d}_{c}")
        else:
            ps = psum.tile([P, 512], F32, tag="po", name=f"ps_{nm}_{d}_{c}")
        for a in range(KT):
            nc.tensor.matmul(
                ps[:, :w],
                w_s[:, d, a, :],
                src[:, a, off : off + w],
                start=(a == 0),
                stop=(a == KT - 1),
            )
        if defer:
            defer.pop(0)()
        nc.vector.tensor_scalar_add(
            dst[:, d, off : off + w], ps[:, :w], bcol[:, d : d + 1]
        )


def _proj_nat(nc, psum, dst, w_s, src, bias_bc, nm):
    """dst[:, i, :] ([P, NT, C] bf16) = src-rows @ W + bias (natural layout).

    src: [P, KT, NV] activations^T -- lhsT tiles [128 contract, 128 seq]
    w_s: [P, KT, C] weight (rhs, contract on partitions)
    bias_bc: [P, C] broadcast bias
    """
    chunks = [(0, 512), (512, 256)]
    for i, (ioff, ih) in enumerate(TILES):
        pss = []
        for (off, w) in chunks:
            ps = psum.tile([P, 512], F32, tag="po", name=f"ps_{nm}_{i}_{off}")
            for a in range(KT):
                nc.tensor.matmul(
                    ps[:ih, :w],
                    src[:, a, ioff : ioff + ih],
                    w_s[:, a, off : off + w],
                    start=(a == 0),
                    stop=(a == KT - 1),
                )
            pss.append(ps)
        for ci, ((off, w), ps) in enumerate(zip(chunks, pss)):
            eng = nc.vector
            eng.tensor_add(out=dst[:ih, i, off : off + w], in0=ps[:ih, :w], in1=bias_bc[:ih, off : off + w])


def _attention(nc, io, psum_main, psum_att, qT, kT, vn, oT, colb, ones_r, ones_row_r, label):
    """oT[:, d, :] = normalized masked-softmax attention output^T ([P, KT, NV] bf16).

    qT, kT: [P, KT, NV] bf16 (d on partitions; scale pre-folded into q).
    vn: [P, NT, C] bf16 (natural).
    colb: [P, NT] f32 = (kmask-1)*10000 along sk partitions (kills pad keys).

    Each chunk's normalization tail (recip bcast matmul + PSUM->SBUF
    copybacks) is DEFERRED and split into parts, emitted one part per
    subsequent PE matmul group, so the PE never head-of-line blocks on the
    DVE recip and the DVE queue never gets one big batch that starves the
    PSUM-ring copybacks. Returns the last chunk's tail parts for the caller
    to spread inside the next phase (via _proj_T/ffn `defer`).
    """
    pend = []
    for c in range(NCH):
        sq = slice(c * FCH, (c + 1) * FCH)
        pos = [
            psum_att.tile([P, FCH], F32, tag="po", name=f"po_{label}_{c}_{d}")
            for d in range(KT)
        ]
        p_tiles = []
        pending_av = []  # av matmuls lag scores by 2 key-tiles so the
        # previous chunk's deferred tail (DVE/Pool copybacks freeing the po
        # banks) completes off the PE critical path

        def av_flush(jj):
            pp = p_tiles[jj]
            jh = TILES[jj][1]
            for d in range(KT):
                nc.tensor.matmul(
                    pos[d][:],
                    vn[:jh, jj, d * P : (d + 1) * P],
                    pp[:jh, :],
                    start=(jj == 0),
                    stop=(jj == NT - 1),
                )

        for j, (joff, jh) in enumerate(TILES):
            st = psum_main.tile([P, 512], F32, tag="st", name=f"st_{label}_{c}_{j}")
            for a in range(KT):
                nc.tensor.matmul(
                    st[:jh, :FCH],
                    kT[:, a, joff : joff + jh],
                    qT[:, a, sq],
                    start=(a == 0),
                    stop=(a == KT - 1),
                )
            if pend:
                pend.pop(0)()
            pj = io.tile([P, FCH], BF16, tag="pp", name=f"p_{label}_{c}_{j}", bufs=NT + 3)
            nc.scalar.activation(pj[:jh, :], st[:jh, :FCH], AF.Exp, bias=colb[:jh, j : j + 1])
            p_tiles.append(pj)
            pending_av.append(j)
            if len(pending_av) > 2:
                av_flush(pending_av.pop(0))
        for jj in pending_av:
            av_flush(jj)
        # row sums over sk (partitions + tiles) via ones-matmul
        rs = psum_main.tile([P, 512], F32, tag="st", name=f"rs_{label}_{c}")
        for j, (joff, jh) in enumerate(TILES):
            nc.tensor.matmul(
                rs[0:1, :FCH],
                ones_r[:jh, :],
                p_tiles[j][:jh, :],
                start=(j == 0),
                stop=(j == NT - 1),
            )
        recip = io.tile([1, FCH], F32R, tag="recip", name=f"recip_{label}_{c}", bufs=2)
        with nc.allow_low_precision(reason="f32r recip feeds f32r bcast matmul"):
            nc.vector.reciprocal(recip[:], rs[0:1, :FCH])

        rbc_box = []

        def tail_bcast(recip=recip, c=c, rbc_box=rbc_box):
            bc = psum_main.tile([P, 512], F32, tag="st", name=f"bc_{label}_{c}")
            nc.tensor.matmul(bc[:, :FCH], ones_row_r[:], recip[:], start=True, stop=True)
            rbc = io.tile([P, FCH], F32, tag="rbc", name=f"rbc_{label}_{c}", bufs=2)
            nc.vector.tensor_copy(out=rbc[:], in_=bc[:, :FCH])
            rbc_box.append(rbc)

        def tail_muls(ds, sq=sq, pos=pos, rbc_box=rbc_box):
            for d in ds:
                nc.vector.tensor_mul(out=oT[:, d, sq], in0=pos[d][:], in1=rbc_box[0][:])

        pend = [tail_bcast] + [
            (lambda ds=ds: tail_muls(ds)) for ds in [(0, 1), (2, 3), (4, 5)]
        ]
    return pend


A2CH = [(0, 256), (256, 256), (512, 64)]  # attn2 query chunks (128-aligned)
A2SUB = [(0, 128), (128, 128), (256, 128), (384, 128), (512, 64)]  # out subtiles


def _attention2_fused(nc, io, psum_main, psum_att, qT, kT, cvf, og_d, colb,
                      rcol, ones_r, ident, bout_bc, defer):
    """Fused attention-2 + ffn: og[q, :] = softmax2(q) @ cvf + bout.

    cvf = t @ (Wcv Wffn) so the attn@v accumulation directly produces the
    final output in NATURAL layout [q part, d' free]; the per-query softmax
    normalization is then a per-partition scalar (rcol), obtained by
    PE-transposing the reciprocal row -- no broadcast matmul, no ffn phase.
    av groups for chunk c are spread across chunk c+1's scores slots.
    """
    p_store = {}  # (c, j) -> p2 tile
    av_queue = []  # pending (c, subtile) av emissions
    fch = [(0, 512), (512, 256)]
    defer = list(defer) if defer else []

    def av_emit(c, t, lo, tw):
        # both free-chunks of one output subtile -> one fin tile, one DMA
        fin = io.tile([P, C], BF16, tag="fin", name=f"fin_{t}", bufs=NT)
        for (off, w) in fch:
            ps = psum_att.tile([P, 512], F32, tag="po", name=f"av2_{t}_{off}")
            for j, (joff, jh) in enumerate(TILES):
                nc.tensor.matmul(
                    ps[:tw, :w],
                    p_store[(c, j)][:jh, lo : lo + tw],
                    cvf[:jh, j, off : off + w],
                    start=(j == 0),
                    stop=(j == NT - 1),
                )
            nc.vector.scalar_tensor_tensor(
                out=fin[:tw, off : off + w],
                in0=ps[:tw, :w],
                scalar=rcol[:tw, t : t + 1],
                in1=bout_bc[:tw, off : off + w],
                op0=AL.mult,
                op1=AL.add,
            )
        eng = nc.scalar if t % 2 == 0 else nc.sync
        eng.dma_start(og_d[t * P : t * P + tw, :], fin[:tw, :])

    def av_pop():
        if av_queue:
            av_queue.pop(0)()

    for c, (qoff, qw) in enumerate(A2CH):
        for j, (joff, jh) in enumerate(TILES):
            st = psum_main.tile([P, 512], F32, tag="st", name=f"st_a2_{c}_{j}")
            for a in range(KT):
                nc.tensor.matmul(
                    st[:jh, :qw],
                    kT[:, a, joff : joff + jh],
                    qT[:, a, qoff : qoff + qw],
                    start=(a == 0),
                    stop=(a == KT - 1),
                )
            if defer:
                defer.pop(0)()
            else:
                av_pop()
            pj = io.tile([P, 256], BF16, tag="pp2", name=f"p2_{c}_{j}", bufs=12)
            nc.scalar.activation(pj[:jh, :qw], st[:jh, :qw], AF.Exp, bias=colb[:jh, j : j + 1])
            p_store[(c, j)] = pj
        # rowsum over keys -> reciprocal row -> PE-transpose to column layout
        rs = psum_main.tile([P, 512], F32, tag="st", name=f"rs_a2_{c}")
        for j, (joff, jh) in enumerate(TILES):
            nc.tensor.matmul(
                rs[0:1, :qw],
                ones_r[:jh, :],
                p_store[(c, j)][:jh, :qw],
                start=(j == 0),
                stop=(j == NT - 1),
            )
        rrow = io.tile([1, 256], F32, tag="rrow", name=f"rrow_{c}", bufs=2)
        nc.vector.reciprocal(rrow[0:1, :qw], rs[0:1, :qw])
        rc_ps = psum_main.tile([P, 512], F32, tag="st", name=f"rcps_{c}")
        subs = [(t, lo, tw) for (t, (g, tw)) in enumerate(A2SUB)
                for lo in [g - qoff] if 0 <= lo < qw]
        for si, (t, lo, tw) in enumerate(subs):
            nc.tensor.transpose(rc_ps[:tw, si : si + 1], rrow[0:1, lo : lo + tw], ident[0:1, 0:1])
        for si, (t, lo, tw) in enumerate(subs):
            nc.vector.tensor_copy(out=rcol[:tw, t : t + 1], in_=rc_ps[:tw, si : si + 1])
        for (t, lo, tw) in subs:
            av_queue.append(lambda c=c, t=t, lo=lo, tw=tw: av_emit(c, t, lo, tw))
    while av_queue:
        av_pop()


def build_nc(n_iters=1):
    nc = bacc.Bacc(trn_type="TRN2", target_bir_lowering=False, debug=False)

    # all big inputs arrive pre-arranged in SBUF layout (host does the
    # (a p) -> p a shuffles) so every DMA row is fully contiguous.
    # T-projection weights additionally carry the out-dim (d) outermost so
    # they can stream per-d-block: [P, d, a, 128].
    xgT_d = nc.dram_tensor("xgT", [P, KT, NV], BF16, kind="ExternalInput").ap()
    tgT_d = nc.dram_tensor("tgT", [P, KT, NV], BF16, kind="ExternalInput").ap()
    colb_ds = {
        nm: nc.dram_tensor(nm, [P, NT], F32, kind="ExternalInput").ap()
        for nm in ["colb1", "colb2"]
    }
    w_ds = {}
    for nm in ["wqk", "wqck"]:
        w_ds[nm] = nc.dram_tensor(nm, [P, KT, KT, P], BF16, kind="ExternalInput").ap()
    for nm in ["wv", "wcvf"]:
        w_ds[nm] = nc.dram_tensor(nm, [P, KT, C], BF16, kind="ExternalInput").ap()
    bcol_ds = {
        nm: nc.dram_tensor(nm, [P, KT], F32, kind="ExternalInput").ap()
        for nm in ["bqk", "bqck"]
    }
    brow_ds = {
        nm: nc.dram_tensor(nm, [1, C], F32, kind="ExternalInput").ap()
        for nm in ["bv", "bcvf", "bf"]
    }
    og_d = nc.dram_tensor("og", [NV, C], BF16, kind="ExternalOutput").ap()

    with tile.TileContext(nc) as tc, ExitStack() as ctx:
        const = ctx.enter_context(tc.tile_pool(name="const", bufs=1))
        acts = ctx.enter_context(tc.tile_pool(name="acts", bufs=1))
        wpool = ctx.enter_context(tc.tile_pool(name="wpool", bufs=1))
        io = ctx.enter_context(tc.tile_pool(name="io", bufs=1))
        psum_main = ctx.enter_context(tc.tile_pool(name="psum_main", bufs=2, space="PSUM"))
        psum_att = ctx.enter_context(tc.tile_pool(name="psum_att", bufs=6, space="PSUM"))

        # ---- constants ----
        # PE warm-up: dummy fp32 matmuls keep the PE busy through the initial
        # DMA head, so the clock is fully ramped (and no pstate penalty
        # applies) when the first real matmuls arrive. The memset source is
        # first in the Pool queue so the warm-up starts ~immediately.
        warm_src = const.tile([P, 256], F32, tag="warm_src", name="warm_src")
        nc.gpsimd.memset(warm_src[:], 1.0)
        warm_ps = psum_main.tile([P, 512], F32, tag="st", name="warm_ps")
        for _wi in range(5):
            nc.tensor.matmul(warm_ps[:, :256], warm_src[:, :P], warm_src[:],
                             start=True, stop=True)

        ones32 = const.tile([P, 1], F32, tag="ones32", name="ones32")
        nc.gpsimd.memset(ones32[:], 1.0)
        ones_r = const.tile([P, 1], BF16, tag="ones_r", name="ones_r")
        nc.vector.tensor_copy(out=ones_r[:], in_=ones32[:])
        ones_row32 = const.tile([1, P], F32, tag="ones_row32", name="ones_row32")
        nc.gpsimd.memset(ones_row32[:], 1.0)
        ones_row_r = const.tile([1, P], F32R, tag="ones_row_r", name="ones_row_r")
        nc.vector.tensor_copy(out=ones_row_r[:], in_=ones_row32[:])

        # const tiles; their DMAs are issued inside the first body iteration,
        # sequenced behind the critical first weight loads
        colbs = {}
        for nm in ["colb1", "colb2"]:
            colbs[nm] = const.tile([P, NT], F32, tag=f"colb_{nm}", name=f"{nm}_s")
        bcols = {}
        for nm in ["bqk", "bqck"]:
            bcols[nm] = const.tile([P, KT], F32, tag=f"bcol_{nm}", name=f"bcol_{nm}")
        brows = {}
        for nm in ["bv", "bcvf", "bf"]:
            brows[nm] = const.tile([P, C], F32, tag=f"brow_{nm}", name=f"brow_{nm}")
        ident32 = const.tile([P, P], F32, tag="ident32", name="ident32")
        make_identity(nc, ident32[:])



        # weight tiles resident in SBUF for the whole kernel; DMAs are issued
        # inside the first body iteration, interleaved in first-use order
        w_ss = {}
        for nm in ["wqk", "wqck"]:
            w_ss[nm] = wpool.tile([P, KT, KT, P], BF16, tag=f"w_{nm}", name=f"ws_{nm}")
        for nm in ["wv", "wcvf"]:
            w_ss[nm] = wpool.tile([P, KT, C], BF16, tag=f"w_{nm}", name=f"ws_{nm}")

        for _it in range(n_iters):
            _body_iter(nc, tc, acts, io, psum_main, psum_att,
                       xgT_d, tgT_d, og_d, w_ds, w_ss, bcols, brows, colbs,
                       bcol_ds, brow_ds, colb_ds, ones_r, ones_row_r, ident32, _it)

    nc.compile()
    return nc


def _body_iter(nc, tc, acts, io, psum_main, psum_att,
               xgT_d, tgT_d, og_d, w_ds, w_ss, bcols, brows, colbs,
               bcol_ds, brow_ds, colb_ds, ones_r, ones_row_r, ident32, it):
    xgT = acts.tile([P, KT, NV], BF16, tag="xgT", name="xgT")
    tgT = acts.tile([P, KT, NV], BF16, tag="tgT", name="tgT")
    if it == 0:
        # single-queue prefetch in exact first-need order; first tiles split
        # so the first projection matmuls start as early as possible
        nc.sync.dma_start(xgT[:, :, :128], xgT_d[:, :, :128])
        nc.sync.dma_start(w_ss["wqk"][:, 0:1], w_ds["wqk"][:, 0:1])
        nc.sync.dma_start(xgT[:, :, 128:FCH], xgT_d[:, :, 128:FCH])
        nc.sync.dma_start(bcols["bqk"][:], bcol_ds["bqk"][:])
        nc.sync.dma_start(xgT[:, :, FCH:], xgT_d[:, :, FCH:])
        nc.sync.dma_start(w_ss["wqk"][:, 1:3], w_ds["wqk"][:, 1:3])
        nc.sync.dma_start(w_ss["wqk"][:, 3:6], w_ds["wqk"][:, 3:6])
        nc.sync.dma_start(w_ss["wv"][:], w_ds["wv"][:])
        nc.sync.dma_start(brows["bv"][:], brow_ds["bv"][0:1, :].partition_broadcast(P))
        nc.sync.dma_start(colbs["colb1"][:], colb_ds["colb1"][:])
        nc.sync.dma_start(tgT[:], tgT_d[:])
        nc.sync.dma_start(w_ss["wqck"][:], w_ds["wqck"][:])
        nc.sync.dma_start(bcols["bqck"][:], bcol_ds["bqck"][:])
        nc.sync.dma_start(colbs["colb2"][:], colb_ds["colb2"][:])
        nc.sync.dma_start(w_ss["wcvf"][:], w_ds["wcvf"][:])
        nc.sync.dma_start(brows["bcvf"][:], brow_ds["bcvf"][0:1, :].partition_broadcast(P))
        nc.sync.dma_start(brows["bf"][:], brow_ds["bf"][0:1, :].partition_broadcast(P))
    else:
        nc.sync.dma_start(xgT[:], xgT_d[:])
        nc.sync.dma_start(tgT[:], tgT_d[:])

    # fused score weights: S1 = x @ (Wq*s @ Wk^T) @ x^T, so attention-1
    # consumes t1 = x @ Wqk as queries and the raw xgT as keys; likewise
    # S2 = o1 @ (Wq*s @ Wck^T) @ t^T. The q/k/cq/ck projections collapse
    # into one projection per attention. Bias terms: per-dim parts fold into
    # bqk/bqck; per-query parts cancel under softmax; per-key parts are
    # folded into colb1/colb2 on the host.
    t1T = acts.tile([P, KT, NV], BF16, tag="qT", name="t1T")
    vn = acts.tile([P, NT, C], BF16, tag="vn", name="vn")
    o1T = acts.tile([P, KT, NV], BF16, tag="oT", name="o1T")

    # ---- phase 1: t1/v projections ----
    # t1 consumes xgT in three pieces matching the DMA arrival order so the
    # first matmul starts after only ~0.5 MB has landed
    _proj_T(nc, psum_att, t1T, w_ss["wqk"], xgT, bcols["bqk"], "t1",
            qchunks=[(0, 128), (128, 160), (288, 288)])
    _proj_nat(nc, psum_att, vn, w_ss["wv"], xgT, brows["bv"], "v")

    # ---- phase 2: attention 1 (keys = raw xgT) ----
    a1_tail = _attention(nc, io, psum_main, psum_att, t1T, xgT, vn, o1T,
                         colbs["colb1"], ones_r, ones_row_r, "a1")

    # ---- phase 3: t2 projection (reuses t1T slot) ----
    t2T = acts.tile([P, KT, NV], BF16, tag="qT", name="t2T")
    _proj_T(nc, psum_att, t2T, w_ss["wqck"], o1T, bcols["bqck"], "t2",
            defer=a1_tail, c_outer=True, psum_first=psum_main, n_first=2)

    # ---- phase 4: cvf projection from text (reuses vn slot) ----
    # cvf = t @ (Wcv Wffn): the ffn is folded into the cross-attn value path
    cvf = acts.tile([P, NT, C], BF16, tag="vn", name="cvf")
    _proj_nat(nc, psum_att, cvf, w_ss["wcvf"], tgT, brows["bcvf"], "cvf")

    # ---- phase 5: fused attention 2 + ffn -> og ----
    rcol = io.tile([P, NT], F32, tag="rcol", name="rcol", bufs=2)
    _attention2_fused(nc, io, psum_main, psum_att, t2T, tgT, cvf, og_d,
                      colbs["colb2"], rcol, ones_r, ident32, brows["bf"],
                      None)


# ---------------- host side ----------------

_NC_CACHE = None


def _get_nc():
    global _NC_CACHE
    if _NC_CACHE is None:
        _NC_CACHE = build_nc()
    return _NC_CACHE


def prepare_static(Wqkv, bqkv, Wq, bq, Wkv, bkv, Wffn, bffn):
    """Shared (per-call, batch-independent) device inputs."""
    s = np.float32(SCALE)
    f32 = np.float32

    def bf(a):  # [C, N] -> [P, KT, N] bf16 with [p, a_, n] = arr[a_*P + p, n]
        a = np.asarray(a)
        return np.ascontiguousarray(
            a.reshape(KT, P, a.shape[1]).transpose(1, 0, 2)
        ).astype(BF16_NP)

    def bf4(a):  # [C, C] -> [P, KT_d, KT_a, P] with [p, d, a_, j] = arr[a_*P+p, d*P+j]
        a = np.asarray(a)
        return np.ascontiguousarray(
            a.reshape(KT, P, KT, P).transpose(1, 2, 0, 3)
        ).astype(BF16_NP)

    def col(b):  # [C] -> [P, KT] with [p, a] = b[a*P + p]
        return np.ascontiguousarray(np.asarray(b, f32).reshape(KT, P).T)

    f64 = np.float64
    wq_s = Wqkv[:, :C].astype(f64) * float(SCALE)
    wk = Wqkv[:, C : 2 * C].astype(f64)
    wcq_s = Wq.astype(f64) * float(SCALE)
    wck = Wkv[:, :C].astype(f64)
    bq1_s = bqkv[:C].astype(f64) * float(SCALE)
    bq2_s = bq.astype(f64) * float(SCALE)

    wf64 = Wffn.astype(f64)
    static = {
        # fused score weights: S1 = x (Wq_s Wk^T) x^T, S2 = o1 (Wcq_s Wck^T) t^T
        "wqk": bf4(wq_s @ wk.T),
        "wqck": bf4(wcq_s @ wck.T),
        "wv": bf(Wqkv[:, 2 * C :]),
        # ffn folded into the cross-attn value path: out = P2 @ (t Wcv Wf) + ...
        "wcvf": bf(Wkv[:, C:].astype(f64) @ wf64),
        # per-dim bias parts of the fused projections
        "bqk": col(bq1_s @ wk.T),
        "bqck": col(bq2_s @ wck.T),
        "bv": np.ascontiguousarray(bqkv[2 * C :], f32).reshape(1, C),
        "bcvf": np.ascontiguousarray(bkv[C:].astype(f64) @ wf64, f32).reshape(1, C),
        "bf": np.ascontiguousarray(bffn, f32).reshape(1, C),
    }
    # per-key score bias directions (keys @ wtilde added to colb on the host;
    # the per-query counterparts cancel under softmax)
    aux = {
        "wt1": (wk @ bq1_s).astype(f32),  # attn1 keys are x rows
        "wt2": (wck @ bq2_s).astype(f32),  # attn2 keys are text rows
    }
    return static, aux


def prepare_core(layout_xb, text_xb, maskb, aux):
    """Per-batch-element compacted device inputs. Returns (in_map, idx) or
    (None, None) if the valid count exceeds NV (host fallback)."""
    idx = np.flatnonzero(maskb != 0)
    nv = len(idx)
    if nv > NV:
        return None, None
    pad_to = idx[0] if nv > 0 else 0
    idxp = np.concatenate([idx, np.full(NV - nv, pad_to, dtype=idx.dtype)])
    km = np.zeros(NT * P, np.float32)  # padded past NV for the colb reshape
    km[:nv] = 1.0
    xg = layout_xb[idxp]
    tg = text_xb[idxp]

    def xf(a):  # [NV, C] gathered rows -> [P, KT, NV] bf16 transposed layout
        return np.ascontiguousarray(
            a.T.reshape(KT, P, NV).transpose(1, 0, 2)
        ).astype(BF16_NP)

    def colb(beta):  # per-key additive score bias incl. pad-kill mask
        v = (km - 1.0) * 10000.0
        v[:NV] += beta
        return np.ascontiguousarray(v.reshape(NT, P).T)

    in_map = {
        "xgT": xf(xg),
        "tgT": xf(tg),
        "colb1": colb(xg.astype(np.float32) @ aux["wt1"]),
        "colb2": colb(tg.astype(np.float32) @ aux["wt2"]),
    }
    return in_map, idx


def masked_row(text_xb, Wkv, bkv, Wffn, bffn):
    """The shared final-output row for all masked positions of one batch
    element: uniform attention over ALL keys -> mean of cross-attn v."""
    mt = text_xb.astype(np.float64).mean(axis=0)
    mcv = mt @ Wkv[:, C:].astype(np.float64) + bkv[C:].astype(np.float64)
    return (mcv @ Wffn.astype(np.float64) + bffn.astype(np.float64)).astype(np.float32)


def _numpy_ref_one(x, t, mask, Wqkv, bqkv, Wq, bq, Wkv, bkv, Wffn, bffn):
    """f64 reference for one batch element (fallback if nv > NV)."""
    x = x.astype(np.float64)
    t = t.astype(np.float64)
    mask = mask.astype(np.float64)
    pair = (mask[:, None] * mask[None, :]) != 0
    scale = C ** -0.5

    def attn(q, k, v):
        sM = (q @ k.T) * scale
        sM = np.where(pair, sM, -10000.0)
        sM = sM - sM.max(axis=-1, keepdims=True)
        e = np.exp(sM)
        return (e / e.sum(axis=-1, keepdims=True)) @ v

    qkv = x @ Wqkv.astype(np.float64) + bqkv.astype(np.float64)
    q, k, v = np.split(qkv, 3, axis=-1)
    lo = attn(q, k, v)
    cq = lo @ Wq.astype(np.float64) + bq.astype(np.float64)
    kv = t @ Wkv.astype(np.float64) + bkv.astype(np.float64)
    ck, cv = np.split(kv, 2, axis=-1)
    mg = attn(cq, ck, cv)
    return (mg @ Wffn.astype(np.float64) + bffn.astype(np.float64)).astype(np.float32)


def kernel(layout_x, text_x, mask, Wqkv, bqkv, Wq, bq, Wkv, bkv, Wffn, bffn):
    layout_x = np.ascontiguousarray(np.asarray(layout_x, dtype=np.float32))
    text_x = np.ascontiguousarray(np.asarray(text_x, dtype=np.float32))
    mask = np.ascontiguousarray(np.asarray(mask, dtype=np.float32))
    Wqkv = np.ascontiguousarray(np.asarray(Wqkv, dtype=np.float32))
    bqkv = np.ascontiguousarray(np.asarray(bqkv, dtype=np.float32)).reshape(3 * C)
    Wq = np.ascontiguousarray(np.asarray(Wq, dtype=np.float32))
    bq = np.ascontiguousarray(np.asarray(bq, dtype=np.float32)).reshape(C)
    Wkv = np.ascontiguousarray(np.asarray(Wkv, dtype=np.float32))
    bkv = np.ascontiguousarray(np.asarray(bkv, dtype=np.float32)).reshape(2 * C)
    Wffn = np.ascontiguousarray(np.asarray(Wffn, dtype=np.float32))
    bffn = np.ascontiguousarray(np.asarray(bffn, dtype=np.float32)).reshape(C)

    B = layout_x.shape[0]
    assert B == N_CORES

    static, aux = prepare_static(Wqkv, bqkv, Wq, bq, Wkv, bkv, Wffn, bffn)
    in_maps, idxs = [], []
    fallback = {}
    for b in range(B):
        in_map, idx = prepare_core(layout_x[b], text_x[b], mask[b], aux)
        if in_map is None:
            fallback[b] = _numpy_ref_one(
                layout_x[b], text_x[b], mask[b],
                Wqkv, bqkv, Wq, bq, Wkv, bkv, Wffn, bffn,
            )
            in_map, idx = prepare_core(
                np.zeros_like(layout_x[b]), np.zeros_like(text_x[b]),
                np.zeros(M, np.float32), aux,
            )
        in_maps.append({**in_map, **static})
        idxs.append(idx)

    nc = _get_nc()
    res = run_bass_kernel_spmd(nc, in_maps, core_ids=list(range(N_CORES)))

    out = np.empty((B, M, C), np.float32)
    for b in range(B):
        if b in fallback:
            out[b] = fallback[b]
            continue
        mrow = masked_row(text_x[b], Wkv, bkv, Wffn, bffn)
        out[b][:] = mrow[None, :]
        idx = idxs[b]
        if len(idx):
            out[b][idx] = res.results[b]["og"][: len(idx)].astype(np.float32)
    return out


# revision 86
# speedup vs baseline: 1.4553x; 1.0380x over previous
"""Trainium2 Bass kernel for nn_Attention_kv (dense transformer block).

Sharding: data-parallel over batch B=8 across the 8 NeuronCores — one batch
element per core, no collectives.

Structural optimizations vs the dense reference:

1. Mask compaction (host): ~50% of positions are masked; every masked QUERY
   row's final output equals ONE shared row per batch element:
       out_masked[b] = (mean_m text_x[b,m] @ Wkv[:,C:] + bkv[C:]) @ Wffn + bffn
   (uniform softmax -> mean of cross-attn v; mean commutes with the linear
   maps). Valid rows attend only to valid keys. The host gathers valid rows
   (padded to static NV=576 >= observed max counts 534/547; overflow falls
   back to a host compute), the device runs a 576-token pipeline, the host
   scatters and fills masked rows.

2. Projection fusion (host algebra): S1 = (xWq)(xWk)^T = x (Wq_s Wk^T) x^T,
   so q/k projections collapse to ONE t1 = x @ Wqk and the raw x^T serves as
   keys; likewise S2 = o1 (Wcq_s Wck^T) t^T. The ffn is folded into the
   cross-attn value path: out = P2 @ (t (Wcv Wffn)) + bias. Bias pieces fold
   into projection biases, cancel under softmax (per-query), or join the
   per-key mask bias colb (host-computed). Softmax scale pre-folded.

3. bf16 PE datapath (1 cycle/row at any width; fp32 PSUM accumulation),
   host pre-transposed/pre-laid-out inputs ([P, a, n] so every DMA row is
   contiguous), single-queue DMA prefetch in exact first-need order.

Per-core pipeline (NV=576, C=768, [part, free] layouts):
  t1 = x@Wqk (^T layout) and v projections
  -> attn1 transposed-scores flash: S^T tile -> exp(S^T + colb1) fused on
     the scalar engine; attn@v accumulated over key tiles in 6 PSUM banks;
     rowsums via ones-matmul; normalization at PSUM->SBUF copyback, its
     recip->bcast->mul tail split into closures spread across later PE work
  -> t2 = o1@Wqck -> cvf = t@(Wcv Wffn)
  -> attn2 fused with ffn: natural-layout output accumulated directly from
     probability tiles against cvf; per-query normalization via PE-transposed
     reciprocal columns (per-partition scalar); og written per subtile.
"""

import sys

sys.path.insert(0, "/opt/trn_rl_repo")

from contextlib import ExitStack

import numpy as np
import ml_dtypes

import concourse.bass as bass
import concourse.mybir as mybir
import concourse.tile as tile
from concourse import bacc
from concourse.bass_utils import run_bass_kernel_spmd
from concourse.masks import make_identity

P = 128
M = 1024  # full sequence length per batch element
C = 768  # model dim
KT = C // P  # 6 contraction tiles
NV = 576  # compacted valid seq len; covers observed max counts 534 (cpu-jax)
# and 547 (axon-jax) with margin; host fallback handles any overflow
NT = 5  # seq tiles: 4 full + one 64-row tail
TILES = [(0, 128), (128, 128), (256, 128), (384, 128), (512, 64)]
FCH = 288  # query free chunk
NCH = NV // FCH  # 2
SCALE = float(C) ** -0.5

F32 = mybir.dt.float32
F32R = mybir.dt.float32r
BF16 = mybir.dt.bfloat16
AF = mybir.ActivationFunctionType
AL = mybir.AluOpType
BF16_NP = ml_dtypes.bfloat16

N_CORES = 8


def _proj_T(nc, psum, dst, w_s, src, bcol, nm, defer=None, c_outer=False,
            qchunks=None, psum_first=None, n_first=0):
    """dst[:, d, :] ([P, KT, NV] bf16) = (src-cols @ W)^T + bias.

    w_s: [P, KT_d, KT_a, P] weight (lhsT tiles [128 contract, 128 out-dim])
    src: [P, KT, NV] activations^T (rhs tiles, contract on partitions)
    bcol: [P, KT] per-out-dim bias columns
    defer: list of closures, one emitted after each matmul group (hides a
    preceding phase's recip->bcast chain behind this phase's PE work)
    c_outer: emit all d-groups of chunk 0 before touching chunk 1 -- use when
    the src's later chunks are produced by the deferred closure
    qchunks: override the free-dim chunk list [(off, w), ...]
    psum_first/n_first: allocate the first n groups' psum from this pool's
    "st" ring instead -- after an attention, the "po" ring's next slots are
    still gated on that attention's normalization chain
    """
    if qchunks is None:
        qchunks = [(c * FCH, FCH) for c in range(NCH)]
    order = (
        [(d, c) for c in range(len(qchunks)) for d in range(KT)]
        if c_outer
        else [(d, c) for d in range(KT) for c in range(len(qchunks))]
    )
    defer = list(defer) if defer else []
    for gi, (d, c) in enumerate(order):
        off, w = qchunks[c]
        if gi < n_first:
            ps = psum_first.tile([P, 512], F32, tag="st", name=f"ps_{nm}_{d}_{c}")
        else:
            ps = psum.tile([P, 512], F32, tag="po", name=f"ps_{nm}_{d}_{c}")
        for a in range(KT):
            nc.tensor.matmul(
                ps[:, :w],
                w_s[:, d, a, :],
                src[:, a, off : off + w],
                start=(a == 0),
                stop=(a == KT - 1),
            )
        if defer:
            defer.pop(0)()
        nc.vector.tensor_scalar_add(
            dst[:, d, off : off + w], ps[:, :w], bcol[:, d : d + 1]
        )


def _proj_nat(nc, psum, dst, w_s, src, bias_bc, nm, defer=None,
              psum_first=None, n_first=0):
    """dst[:, i, :] ([P, NT, C] bf16) = src-rows @ W + bias (natural layout).

    src: [P, KT, NV] activations^T -- lhsT tiles [128 contract, 128 seq]
    w_s: [P, KT, C] weight (rhs, contract on partitions)
    bias_bc: [P, C] broadcast bias
    defer/psum_first/n_first: as in _proj_T (spread a preceding attention's
    normalization tail; keep early groups off the just-recycled po ring)
    """
    chunks = [(0, 512), (512, 256)]
    defer = list(defer) if defer else []
    gi = 0
    for i, (ioff, ih) in enumerate(TILES):
        pss = []
        for (off, w) in chunks:
            if gi < n_first:
                ps = psum_first.tile([P, 512], F32, tag="st", name=f"ps_{nm}_{i}_{off}")
            else:
                ps = psum.tile([P, 512], F32, tag="po", name=f"ps_{nm}_{i}_{off}")
            gi += 1
            for a in range(KT):
                nc.tensor.matmul(
                    ps[:ih, :w],
                    src[:, a, ioff : ioff + ih],
                    w_s[:, a, off : off + w],
                    start=(a == 0),
                    stop=(a == KT - 1),
                )
            if defer:
                defer.pop(0)()
            pss.append(ps)
        for ci, ((off, w), ps) in enumerate(zip(chunks, pss)):
            eng = nc.vector
            eng.tensor_add(out=dst[:ih, i, off : off + w], in0=ps[:ih, :w], in1=bias_bc[:ih, off : off + w])


def _attention(nc, io, psum_main, psum_att, qT, kT, vn, oT, colb, ones_r, ones_row_r, label):
    """oT[:, d, :] = normalized masked-softmax attention output^T ([P, KT, NV] bf16).

    qT, kT: [P, KT, NV] bf16 (d on partitions; scale pre-folded into q).
    vn: [P, NT, C] bf16 (natural).
    colb: [P, NT] f32 = (kmask-1)*10000 along sk partitions (kills pad keys).

    Each chunk's normalization tail (recip bcast matmul + PSUM->SBUF
    copybacks) is DEFERRED and split into parts, emitted one part per
    subsequent PE matmul group, so the PE never head-of-line blocks on the
    DVE recip and the DVE queue never gets one big batch that starves the
    PSUM-ring copybacks. Returns the last chunk's tail parts for the caller
    to spread inside the next phase (via _proj_T/ffn `defer`).
    """
    pend = []
    for c in range(NCH):
        sq = slice(c * FCH, (c + 1) * FCH)
        pos = [
            psum_att.tile([P, FCH], F32, tag="po", name=f"po_{label}_{c}_{d}")
            for d in range(KT)
        ]
        p_tiles = []
        pending_av = []  # av matmuls lag scores by 2 key-tiles so the
        # previous chunk's deferred tail (DVE/Pool copybacks freeing the po
        # banks) completes off the PE critical path

        def av_flush(jj):
            pp = p_tiles[jj]
            jh = TILES[jj][1]
            for d in range(KT):
                nc.tensor.matmul(
                    pos[d][:],
                    vn[:jh, jj, d * P : (d + 1) * P],
                    pp[:jh, :],
                    start=(jj == 0),
                    stop=(jj == NT - 1),
                )

        for j, (joff, jh) in enumerate(TILES):
            st = psum_main.tile([P, 512], F32, tag="st", name=f"st_{label}_{c}_{j}")
            for a in range(KT):
                nc.tensor.matmul(
                    st[:jh, :FCH],
                    kT[:, a, joff : joff + jh],
                    qT[:, a, sq],
                    start=(a == 0),
                    stop=(a == KT - 1),
                )
            if pend:
                pend.pop(0)()
            pj = io.tile([P, FCH], BF16, tag="pp", name=f"p_{label}_{c}_{j}", bufs=NT + 3)
            nc.scalar.activation(pj[:jh, :], st[:jh, :FCH], AF.Exp, bias=colb[:jh, j : j + 1])
            p_tiles.append(pj)
            pending_av.append(j)
            if len(pending_av) > 2:
                av_flush(pending_av.pop(0))
        for jj in pending_av:
            av_flush(jj)
        # row sums over sk (partitions + tiles) via ones-matmul
        rs = psum_main.tile([P, 512], F32, tag="st", name=f"rs_{label}_{c}")
        for j, (joff, jh) in enumerate(TILES):
            nc.tensor.matmul(
                rs[0:1, :FCH],
                ones_r[:jh, :],
                p_tiles[j][:jh, :],
                start=(j == 0),
                stop=(j == NT - 1),
            )
        recip = io.tile([1, FCH], F32R, tag="recip", name=f"recip_{label}_{c}", bufs=2)
        with nc.allow_low_precision(reason="f32r recip feeds f32r bcast matmul"):
            nc.vector.reciprocal(recip[:], rs[0:1, :FCH])

        rbc_box = []

        def tail_bcast(recip=recip, c=c, rbc_box=rbc_box):
            bc = psum_main.tile([P, 512], F32, tag="st", name=f"bc_{label}_{c}")
            nc.tensor.matmul(bc[:, :FCH], ones_row_r[:], recip[:], start=True, stop=True)
            rbc = io.tile([P, FCH], F32, tag="rbc", name=f"rbc_{label}_{c}", bufs=2)
            nc.vector.tensor_copy(out=rbc[:], in_=bc[:, :FCH])
            rbc_box.append(rbc)

        def tail_muls(ds, sq=sq, pos=pos, rbc_box=rbc_box):
            for d in ds:
                nc.vector.tensor_mul(out=oT[:, d, sq], in0=pos[d][:], in1=rbc_box[0][:])

        pend = [tail_bcast] + [
            (lambda ds=ds: tail_muls(ds)) for ds in [(0, 1), (2, 3), (4, 5)]
        ]
    return pend


A2CH = [(0, 256), (256, 256), (512, 64)]  # attn2 query chunks (128-aligned)
A2SUB = [(0, 128), (128, 128), (256, 128), (384, 128), (512, 64)]  # out subtiles


def _attention2_fused(nc, io, psum_main, psum_att, qT, kT, cvf, og_d, colb,
                      rcol, ones_r, ident, bout_bc, defer):
    """Fused attention-2 + ffn: og[q, :] = softmax2(q) @ cvf + bout.

    cvf = t @ (Wcv Wffn) so the attn@v accumulation directly produces the
    final output in NATURAL layout [q part, d' free]; the per-query softmax
    normalization is then a per-partition scalar (rcol), obtained by
    PE-transposing the reciprocal row -- no broadcast matmul, no ffn phase.
    av groups for chunk c are spread across chunk c+1's scores slots.
    """
    p_store = {}  # (c, j) -> p2 tile
    av_queue = []  # pending (c, subtile) av emissions
    fch = [(0, 512), (512, 256)]
    defer = list(defer) if defer else []

    def av_emit(c, t, lo, tw):
        # both free-chunks of one output subtile -> one fin tile, one DMA
        fin = io.tile([P, C], BF16, tag="fin", name=f"fin_{t}", bufs=NT)
        for (off, w) in fch:
            ps = psum_att.tile([P, 512], F32, tag="po", name=f"av2_{t}_{off}")
            for j, (joff, jh) in enumerate(TILES):
                nc.tensor.matmul(
                    ps[:tw, :w],
                    p_store[(c, j)][:jh, lo : lo + tw],
                    cvf[:jh, j, off : off + w],
                    start=(j == 0),
                    stop=(j == NT - 1),
                )
            nc.vector.scalar_tensor_tensor(
                out=fin[:tw, off : off + w],
                in0=ps[:tw, :w],
                scalar=rcol[:tw, t : t + 1],
                in1=bout_bc[:tw, off : off + w],
                op0=AL.mult,
                op1=AL.add,
            )
        eng = nc.scalar if t % 2 == 0 else nc.sync
        eng.dma_start(og_d[t * P : t * P + tw, :], fin[:tw, :])

    def av_pop():
        if av_queue:
            av_queue.pop(0)()

    for c, (qoff, qw) in enumerate(A2CH):
        for j, (joff, jh) in enumerate(TILES):
            st = psum_main.tile([P, 512], F32, tag="st", name=f"st_a2_{c}_{j}")
            for a in range(KT):
                nc.tensor.matmul(
                    st[:jh, :qw],
                    kT[:, a, joff : joff + jh],
                    qT[:, a, qoff : qoff + qw],
                    start=(a == 0),
                    stop=(a == KT - 1),
                )
            if defer:
                defer.pop(0)()
            else:
                av_pop()
            pj = io.tile([P, 256], BF16, tag="pp2", name=f"p2_{c}_{j}", bufs=12)
            nc.scalar.activation(pj[:jh, :qw], st[:jh, :qw], AF.Exp, bias=colb[:jh, j : j + 1])
            p_store[(c, j)] = pj
        # rowsum over keys -> reciprocal row -> PE-transpose to column layout
        rs = psum_main.tile([P, 512], F32, tag="st", name=f"rs_a2_{c}")
        for j, (joff, jh) in enumerate(TILES):
            nc.tensor.matmul(
                rs[0:1, :qw],
                ones_r[:jh, :],
                p_store[(c, j)][:jh, :qw],
                start=(j == 0),
                stop=(j == NT - 1),
            )
        rrow = io.tile([1, 256], F32, tag="rrow", name=f"rrow_{c}", bufs=2)
        nc.vector.reciprocal(rrow[0:1, :qw], rs[0:1, :qw])
        rc_ps = psum_main.tile([P, 512], F32, tag="st", name=f"rcps_{c}")
        subs = [(t, lo, tw) for (t, (g, tw)) in enumerate(A2SUB)
                for lo in [g - qoff] if 0 <= lo < qw]
        for si, (t, lo, tw) in enumerate(subs):
            nc.tensor.transpose(rc_ps[:tw, si : si + 1], rrow[0:1, lo : lo + tw], ident[0:1, 0:1])
        for si, (t, lo, tw) in enumerate(subs):
            nc.vector.tensor_copy(out=rcol[:tw, t : t + 1], in_=rc_ps[:tw, si : si + 1])
        for (t, lo, tw) in subs:
            av_queue.append(lambda c=c, t=t, lo=lo, tw=tw: av_emit(c, t, lo, tw))
    while av_queue:
        av_pop()


def build_nc(n_iters=1):
    nc = bacc.Bacc(trn_type="TRN2", target_bir_lowering=False, debug=False)

    # all big inputs arrive pre-arranged in SBUF layout (host does the
    # (a p) -> p a shuffles) so every DMA row is fully contiguous.
    # T-projection weights additionally carry the out-dim (d) outermost so
    # they can stream per-d-block: [P, d, a, 128].
    xgT_d = nc.dram_tensor("xgT", [P, KT, NV], BF16, kind="ExternalInput").ap()
    tgT_d = nc.dram_tensor("tgT", [P, KT, NV], BF16, kind="ExternalInput").ap()
    colb_ds = {
        nm: nc.dram_tensor(nm, [P, NT], F32, kind="ExternalInput").ap()
        for nm in ["colb1", "colb2"]
    }
    w_ds = {}
    for nm in ["wqk"]:
        w_ds[nm] = nc.dram_tensor(nm, [P, KT, KT, P], BF16, kind="ExternalInput").ap()
    for nm in ["wv", "wcvf"]:
        w_ds[nm] = nc.dram_tensor(nm, [P, KT, C], BF16, kind="ExternalInput").ap()
    bcol_ds = {
        nm: nc.dram_tensor(nm, [P, KT], F32, kind="ExternalInput").ap()
        for nm in ["bqk"]
    }
    brow_ds = {
        nm: nc.dram_tensor(nm, [1, C], F32, kind="ExternalInput").ap()
        for nm in ["bv", "bcvf", "bf"]
    }
    og_d = nc.dram_tensor("og", [NV, C], BF16, kind="ExternalOutput").ap()

    with tile.TileContext(nc) as tc, ExitStack() as ctx:
        const = ctx.enter_context(tc.tile_pool(name="const", bufs=1))
        acts = ctx.enter_context(tc.tile_pool(name="acts", bufs=1))
        wpool = ctx.enter_context(tc.tile_pool(name="wpool", bufs=1))
        io = ctx.enter_context(tc.tile_pool(name="io", bufs=1))
        psum_main = ctx.enter_context(tc.tile_pool(name="psum_main", bufs=2, space="PSUM"))
        psum_att = ctx.enter_context(tc.tile_pool(name="psum_att", bufs=6, space="PSUM"))

        # ---- constants ----
        # PE warm-up: dummy fp32 matmuls keep the PE busy through the initial
        # DMA head, so the clock is fully ramped (and no pstate penalty
        # applies) when the first real matmuls arrive. The memset source is
        # first in the Pool queue so the warm-up starts ~immediately.
        warm_src = const.tile([P, 256], F32, tag="warm_src", name="warm_src")
        nc.gpsimd.memset(warm_src[:], 1.0)
        warm_ps = psum_main.tile([P, 512], F32, tag="st", name="warm_ps")
        for _wi in range(5):
            nc.tensor.matmul(warm_ps[:, :256], warm_src[:, :P], warm_src[:],
                             start=True, stop=True)

        ones32 = const.tile([P, 1], F32, tag="ones32", name="ones32")
        nc.gpsimd.memset(ones32[:], 1.0)
        ones_r = const.tile([P, 1], BF16, tag="ones_r", name="ones_r")
        nc.vector.tensor_copy(out=ones_r[:], in_=ones32[:])
        ones_row32 = const.tile([1, P], F32, tag="ones_row32", name="ones_row32")
        nc.gpsimd.memset(ones_row32[:], 1.0)
        ones_row_r = const.tile([1, P], F32R, tag="ones_row_r", name="ones_row_r")
        nc.vector.tensor_copy(out=ones_row_r[:], in_=ones_row32[:])

        # const tiles; their DMAs are issued inside the first body iteration,
        # sequenced behind the critical first weight loads
        colbs = {}
        for nm in ["colb1", "colb2"]:
            colbs[nm] = const.tile([P, NT], F32, tag=f"colb_{nm}", name=f"{nm}_s")
        bcols = {}
        for nm in ["bqk"]:
            bcols[nm] = const.tile([P, KT], F32, tag=f"bcol_{nm}", name=f"bcol_{nm}")
        brows = {}
        for nm in ["bv", "bcvf", "bf"]:
            brows[nm] = const.tile([P, C], F32, tag=f"brow_{nm}", name=f"brow_{nm}")
        ident32 = const.tile([P, P], F32, tag="ident32", name="ident32")
        make_identity(nc, ident32[:])



        # weight tiles resident in SBUF for the whole kernel; DMAs are issued
        # inside the first body iteration, interleaved in first-use order
        w_ss = {}
        for nm in ["wqk"]:
            w_ss[nm] = wpool.tile([P, KT, KT, P], BF16, tag=f"w_{nm}", name=f"ws_{nm}")
        for nm in ["wv", "wcvf"]:
            w_ss[nm] = wpool.tile([P, KT, C], BF16, tag=f"w_{nm}", name=f"ws_{nm}")

        for _it in range(n_iters):
            _body_iter(nc, tc, acts, io, psum_main, psum_att,
                       xgT_d, tgT_d, og_d, w_ds, w_ss, bcols, brows, colbs,
                       bcol_ds, brow_ds, colb_ds, ones_r, ones_row_r, ident32, _it)

    nc.compile()
    return nc


def _body_iter(nc, tc, acts, io, psum_main, psum_att,
               xgT_d, tgT_d, og_d, w_ds, w_ss, bcols, brows, colbs,
               bcol_ds, brow_ds, colb_ds, ones_r, ones_row_r, ident32, it):
    xgT = acts.tile([P, KT, NV], BF16, tag="xgT", name="xgT")
    tgT = acts.tile([P, KT, NV], BF16, tag="tgT", name="tgT")
    if it == 0:
        # single-queue prefetch in exact first-need order; first tiles split
        # so the first projection matmuls start as early as possible
        nc.sync.dma_start(xgT[:, :, :128], xgT_d[:, :, :128])
        nc.sync.dma_start(w_ss["wqk"][:, 0:1], w_ds["wqk"][:, 0:1])
        nc.sync.dma_start(xgT[:, :, 128:FCH], xgT_d[:, :, 128:FCH])
        nc.sync.dma_start(bcols["bqk"][:], bcol_ds["bqk"][:])
        nc.sync.dma_start(xgT[:, :, FCH:], xgT_d[:, :, FCH:])
        nc.sync.dma_start(w_ss["wqk"][:, 1:3], w_ds["wqk"][:, 1:3])
        nc.sync.dma_start(w_ss["wqk"][:, 3:6], w_ds["wqk"][:, 3:6])
        nc.sync.dma_start(w_ss["wv"][:], w_ds["wv"][:])
        nc.sync.dma_start(brows["bv"][:], brow_ds["bv"][0:1, :].partition_broadcast(P))
        nc.sync.dma_start(colbs["colb1"][:], colb_ds["colb1"][:])
        nc.sync.dma_start(tgT[:], tgT_d[:])
        nc.sync.dma_start(colbs["colb2"][:], colb_ds["colb2"][:])
        nc.sync.dma_start(w_ss["wcvf"][:], w_ds["wcvf"][:])
        nc.sync.dma_start(brows["bcvf"][:], brow_ds["bcvf"][0:1, :].partition_broadcast(P))
        nc.sync.dma_start(brows["bf"][:], brow_ds["bf"][0:1, :].partition_broadcast(P))
    else:
        nc.sync.dma_start(xgT[:], xgT_d[:])
        nc.sync.dma_start(tgT[:], tgT_d[:])

    # fused score weights: S1 = x @ (Wq*s @ Wk^T) @ x^T, so attention-1
    # consumes t1 = x @ Wqk as queries and the raw xgT as keys. The t2
    # projection is absorbed into attention-1's value path:
    #   t2 = P1n @ v @ Wqck = P1n @ (x @ (Wv Wqck))
    # so "wv"/"bv" hold the host-fused Wv@Wqck product and attention-1's
    # normalized copyback writes t2 directly. Remaining bias terms fold into
    # bqk / colb1 / colb2 on the host; per-query parts cancel under softmax.
    t1T = acts.tile([P, KT, NV], BF16, tag="qT", name="t1T")
    vqn = acts.tile([P, NT, C], BF16, tag="vn", name="vqn")
    t2T = acts.tile([P, KT, NV], BF16, tag="oT", name="t2T")

    # ---- phase 1: t1/vq projections ----
    # t1 consumes xgT in three pieces matching the DMA arrival order so the
    # first matmul starts after only ~0.5 MB has landed
    _proj_T(nc, psum_att, t1T, w_ss["wqk"], xgT, bcols["bqk"], "t1",
            qchunks=[(0, 128), (128, 160), (288, 288)])
    _proj_nat(nc, psum_att, vqn, w_ss["wv"], xgT, brows["bv"], "vq")

    # ---- phase 2: attention 1 (keys = raw xgT) -> t2T directly ----
    a1_tail = _attention(nc, io, psum_main, psum_att, t1T, xgT, vqn, t2T,
                         colbs["colb1"], ones_r, ones_row_r, "a1")

    # ---- phase 3: cvf projection from text (reuses vn slot) ----
    # cvf = t @ (Wcv Wffn): the ffn is folded into the cross-attn value path
    cvf = acts.tile([P, NT, C], BF16, tag="vn", name="cvf")
    _proj_nat(nc, psum_att, cvf, w_ss["wcvf"], tgT, brows["bcvf"], "cvf",
              defer=a1_tail, psum_first=psum_main, n_first=2)

    # ---- phase 5: fused attention 2 + ffn -> og ----
    rcol = io.tile([P, NT], F32, tag="rcol", name="rcol", bufs=2)
    _attention2_fused(nc, io, psum_main, psum_att, t2T, tgT, cvf, og_d,
                      colbs["colb2"], rcol, ones_r, ident32, brows["bf"],
                      None)


# ---------------- host side ----------------

_NC_CACHE = None


def _get_nc():
    global _NC_CACHE
    if _NC_CACHE is None:
        _NC_CACHE = build_nc()
    return _NC_CACHE


def prepare_static(Wqkv, bqkv, Wq, bq, Wkv, bkv, Wffn, bffn):
    """Shared (per-call, batch-independent) device inputs."""
    s = np.float32(SCALE)
    f32 = np.float32

    def bf(a):  # [C, N] -> [P, KT, N] bf16 with [p, a_, n] = arr[a_*P + p, n]
        a = np.asarray(a)
        return np.ascontiguousarray(
            a.reshape(KT, P, a.shape[1]).transpose(1, 0, 2)
        ).astype(BF16_NP)

    def bf4(a):  # [C, C] -> [P, KT_d, KT_a, P] with [p, d, a_, j] = arr[a_*P+p, d*P+j]
        a = np.asarray(a)
        return np.ascontiguousarray(
            a.reshape(KT, P, KT, P).transpose(1, 2, 0, 3)
        ).astype(BF16_NP)

    def col(b):  # [C] -> [P, KT] with [p, a] = b[a*P + p]
        return np.ascontiguousarray(np.asarray(b, f32).reshape(KT, P).T)

    f64 = np.float64
    wq_s = Wqkv[:, :C].astype(f64) * float(SCALE)
    wk = Wqkv[:, C : 2 * C].astype(f64)
    wcq_s = Wq.astype(f64) * float(SCALE)
    wck = Wkv[:, :C].astype(f64)
    bq1_s = bqkv[:C].astype(f64) * float(SCALE)
    bq2_s = bq.astype(f64) * float(SCALE)

    wf64 = Wffn.astype(f64)
    wqck = wcq_s @ wck.T  # fused attn2 score weight (Wq*s) Wck^T
    static = {
        # fused score weights: S1 = x (Wq_s Wk^T) x^T, S2 = t2 tg^T
        "wqk": bf4(wq_s @ wk.T),
        # t2 projection absorbed into attention-1's value path:
        # t2 = P1n @ (x @ (Wv Wqck)) (+ bv Wqck via the projection bias)
        "wv": bf(Wqkv[:, 2 * C :].astype(f64) @ wqck),
        # ffn folded into the cross-attn value path: out = P2 @ (t Wcv Wf) + ...
        "wcvf": bf(Wkv[:, C:].astype(f64) @ wf64),
        # per-dim bias parts of the fused projections
        "bqk": col(bq1_s @ wk.T),
        "bv": np.ascontiguousarray(bqkv[2 * C :].astype(f64) @ wqck, f32).reshape(1, C),
        "bcvf": np.ascontiguousarray(bkv[C:].astype(f64) @ wf64, f32).reshape(1, C),
        "bf": np.ascontiguousarray(bffn, f32).reshape(1, C),
    }
    # per-key score bias for attn2 (keys=text): the cq-side bias term
    # bq2_s . ck[j] = tg[j] @ (Wck bq2_s), added to colb2 on the host.
    # (attn1's counterpart is carried by the bqk projection bias instead;
    # per-query terms cancel under softmax.)
    aux = {
        "wt2": (wck @ bq2_s).astype(f32),
    }
    return static, aux


def prepare_core(layout_xb, text_xb, maskb, aux):
    """Per-batch-element compacted device inputs. Returns (in_map, idx) or
    (None, None) if the valid count exceeds NV (host fallback)."""
    idx = np.flatnonzero(maskb != 0)
    nv = len(idx)
    if nv > NV:
        return None, None
    pad_to = idx[0] if nv > 0 else 0
    idxp = np.concatenate([idx, np.full(NV - nv, pad_to, dtype=idx.dtype)])
    km = np.zeros(NT * P, np.float32)  # padded past NV for the colb reshape
    km[:nv] = 1.0
    xg = layout_xb[idxp]
    tg = text_xb[idxp]

    def xf(a):  # [NV, C] gathered rows -> [P, KT, NV] bf16 transposed layout
        return np.ascontiguousarray(
            a.T.reshape(KT, P, NV).transpose(1, 0, 2)
        ).astype(BF16_NP)

    def colb(beta):  # per-key additive score bias incl. pad-kill mask
        v = (km - 1.0) * 10000.0
        v[:NV] += beta
        return np.ascontiguousarray(v.reshape(NT, P).T)

    in_map = {
        "xgT": xf(xg),
        "tgT": xf(tg),
        "colb1": colb(np.zeros(NV, np.float32)),
        "colb2": colb(tg.astype(np.float32) @ aux["wt2"]),
    }
    return in_map, idx


def masked_row(text_xb, Wkv, bkv, Wffn, bffn):
    """The shared final-output row for all masked positions of one batch
    element: uniform attention over ALL keys -> mean of cross-attn v."""
    mt = text_xb.astype(np.float64).mean(axis=0)
    mcv = mt @ Wkv[:, C:].astype(np.float64) + bkv[C:].astype(np.float64)
    return (mcv @ Wffn.astype(np.float64) + bffn.astype(np.float64)).astype(np.float32)


def _numpy_ref_one(x, t, mask, Wqkv, bqkv, Wq, bq, Wkv, bkv, Wffn, bffn):
    """f64 reference for one batch element (fallback if nv > NV)."""
    x = x.astype(np.float64)
    t = t.astype(np.float64)
    mask = mask.astype(np.float64)
    pair = (mask[:, None] * mask[None, :]) != 0
    scale = C ** -0.5

    def attn(q, k, v):
        sM = (q @ k.T) * scale
        sM = np.where(pair, sM, -10000.0)
        sM = sM - sM.max(axis=-1, keepdims=True)
        e = np.exp(sM)
        return (e / e.sum(axis=-1, keepdims=True)) @ v

    qkv = x @ Wqkv.astype(np.float64) + bqkv.astype(np.float64)
    q, k, v = np.split(qkv, 3, axis=-1)
    lo = attn(q, k, v)
    cq = lo @ Wq.astype(np.float64) + bq.astype(np.float64)
    kv = t @ Wkv.astype(np.float64) + bkv.astype(np.float64)
    ck, cv = np.split(kv, 2, axis=-1)
    mg = attn(cq, ck, cv)
    return (mg @ Wffn.astype(np.float64) + bffn.astype(np.float64)).astype(np.float32)


def kernel(layout_x, text_x, mask, Wqkv, bqkv, Wq, bq, Wkv, bkv, Wffn, bffn):
    layout_x = np.ascontiguousarray(np.asarray(layout_x, dtype=np.float32))
    text_x = np.ascontiguousarray(np.asarray(text_x, dtype=np.float32))
    mask = np.ascontiguousarray(np.asarray(mask, dtype=np.float32))
    Wqkv = np.ascontiguousarray(np.asarray(Wqkv, dtype=np.float32))
    bqkv = np.ascontiguousarray(np.asarray(bqkv, dtype=np.float32)).reshape(3 * C)
    Wq = np.ascontiguousarray(np.asarray(Wq, dtype=np.float32))
    bq = np.ascontiguousarray(np.asarray(bq, dtype=np.float32)).reshape(C)
    Wkv = np.ascontiguousarray(np.asarray(Wkv, dtype=np.float32))
    bkv = np.ascontiguousarray(np.asarray(bkv, dtype=np.float32)).reshape(2 * C)
    Wffn = np.ascontiguousarray(np.asarray(Wffn, dtype=np.float32))
    bffn = np.ascontiguousarray(np.asarray(bffn, dtype=np.float32)).reshape(C)

    B = layout_x.shape[0]
    assert B == N_CORES

    static, aux = prepare_static(Wqkv, bqkv, Wq, bq, Wkv, bkv, Wffn, bffn)
    in_maps, idxs = [], []
    fallback = {}
    for b in range(B):
        in_map, idx = prepare_core(layout_x[b], text_x[b], mask[b], aux)
        if in_map is None:
            fallback[b] = _numpy_ref_one(
                layout_x[b], text_x[b], mask[b],
                Wqkv, bqkv, Wq, bq, Wkv, bkv, Wffn, bffn,
            )
            in_map, idx = prepare_core(
                np.zeros_like(layout_x[b]), np.zeros_like(text_x[b]),
                np.zeros(M, np.float32), aux,
            )
        in_maps.append({**in_map, **static})
        idxs.append(idx)

    nc = _get_nc()
    res = run_bass_kernel_spmd(nc, in_maps, core_ids=list(range(N_CORES)))

    out = np.empty((B, M, C), np.float32)
    for b in range(B):
        if b in fallback:
            out[b] = fallback[b]
            continue
        mrow = masked_row(text_x[b], Wkv, bkv, Wffn, bffn)
        out[b][:] = mrow[None, :]
        idx = idxs[b]
        if len(idx):
            out[b][idx] = res.results[b]["og"][: len(idx)].astype(np.float32)
    return out
